# revision 1
# baseline (speedup 1.0000x reference)
"""Bass/Tile TRN2 kernel for retrieval-KNN MSE distance matrix.

Computes: out = ||t||^2 + ||s@W.T+b||^2 - 2 * t @ (s@W.T+b).T   [N=4096, M=4096]

Sharding (8 cores, output column-parallel, no collectives):
  core c holds s_rep rows [c*512, (c+1)*512) and computes the full-height
  output block out[:, c*512:(c+1)*512].  Per-core work:
    GEMM1: s_projT[1536, 512] = WT.T @ sT         (K-major, 12x12 k/m chunks)
    GEMM2: out_j[128, 512]    = tT_j.T @ s_projT  accumulated over 12 k-chunks
  t_sq comes from gram-block matmuls (tile.T @ tile, diagonal extracted via
  identity mask + tensor_tensor_reduce) and enters as the fp32 ACT bias;
  s_sq comes from a ones-matmul over the squared projection and is folded
  into PSUM with a compensated K=2 matmul (hi+lo bf16 split, scaled -0.5)
  so the ACT copyback (scale=-2, bias=t_sq) produces the final value.

Matmuls run in bf16 (fp32 inputs cast on-chip by DVE); accumulation fp32.
"""

import numpy as np

import concourse.bacc as bacc
import concourse.bass as bass
import concourse.mybir as mybir
import concourse.tile as tile
from concourse.bass_utils import run_bass_kernel_spmd

N = 4096          # t_rep rows
M = 4096          # s_rep rows
D = 1536          # feature dim
NCORES = 8
MC = M // NCORES  # 512: output columns per core
KC = D // 128     # 12:  contraction chunks
NJ = N // 128     # 32:  output row chunks per core

FP32 = mybir.dt.float32
BF16 = mybir.dt.bfloat16
AF = mybir.ActivationFunctionType


def build_nc(variant="full"):
    nc = bacc.Bacc("TRN2", target_bir_lowering=False, num_devices=NCORES)

    t_in = nc.dram_tensor("t", [NJ // 4, KC, 128, 512], FP32, kind="ExternalInput").ap()
    s_in = nc.dram_tensor("s", [KC, 128, MC], FP32, kind="ExternalInput").ap()
    w_in = nc.dram_tensor("w", [KC, 128, D], FP32, kind="ExternalInput").ap()
    b_in = nc.dram_tensor("b", [KC, 128, 1], FP32, kind="ExternalInput").ap()
    id_in = nc.dram_tensor("ident", [128, 128], FP32, kind="ExternalInput").ap()
    out = nc.dram_tensor("out", [NJ, 128, MC], FP32, kind="ExternalOutput").ap()

    with tile.TileContext(nc) as tc:
        with (
            tc.tile_pool(name="const", bufs=1) as const_pool,
            tc.tile_pool(name="sproj", bufs=1) as sproj_pool,
            tc.tile_pool(name="small", bufs=1) as small_pool,
            tc.tile_pool(name="psum_main", bufs=4, space="PSUM") as psum_main,
        ):
            ident = const_pool.tile([128, 128], FP32)
            nc.sync.dma_start(out=ident[:], in_=id_in[:, :])
            ones_col = const_pool.tile([128, 1], BF16)  # lhsT for s_sq row-matmul
            nc.vector.memset(ones_col[:], 1.0)

            # ---- Phase 0: HAM warmup while initial DMAs stream ----
            warm = const_pool.tile([128, MC], BF16, name="warm")
            nc.vector.memset(warm[:], 0.5)
            with tc.tile_pool(name="psum_warm", bufs=1, space="PSUM") as pw_pool:
                pw = pw_pool.tile([128, MC], FP32, name="pw")
                for i in range(60):
                    nc.tensor.matmul(pw[:], lhsT=warm[:, 0:128], rhs=warm[:],
                                     start=(i == 0), stop=(i == 59))

            # ---- Phase 1: projection s_projT[d, r] + bias, and s_sq ----
            sproj = []  # 12 tiles [128, MC] bf16
            with (
                tc.tile_pool(name="wts", bufs=6) as wt_pool,
                tc.tile_pool(name="wtb", bufs=1) as wtb_pool,
                tc.tile_pool(name="srep", bufs=2) as s_pool,
                tc.tile_pool(name="srepb", bufs=1) as sb_pool,
                tc.tile_pool(name="bias", bufs=1) as b_pool,
                tc.tile_pool(name="sq", bufs=3) as sq_pool,
                tc.tile_pool(name="psum_aux", bufs=2, space="PSUM") as psum_aux,
            ):
                wt_sb = []
                s_sb = []
                b_sb = []
                wt_f32 = []
                for k in range(KC):
                    st = s_pool.tile([128, MC], FP32, name="st")
                    nc.sync.dma_start(out=st[:], in_=s_in[k])
                    stb = sb_pool.tile([128, MC], BF16, name=f"stb{k}")
                    nc.vector.tensor_copy(stb[:], st[:])
                    s_sb.append(stb)

                    bt = b_pool.tile([128, 1], FP32, name=f"bt{k}")
                    nc.sync.dma_start(out=bt[:], in_=b_in[k])
                    b_sb.append(bt)

                    wt_sb.append(wtb_pool.tile([128, D], BF16, name=f"wtb{k}"))
                # column-group-major W loads through small fp32 piece tiles:
                # GEMM1 j-block c can start after the first 12 pieces land
                for c in range(D // 512):
                    for k in range(KC):
                        sl = slice(c * 512, (c + 1) * 512)
                        wtp = wt_pool.tile([128, 512], FP32, name="wtp")
                        nc.sync.dma_start(out=wtp[:], in_=w_in[k][:, sl])
                        nc.vector.tensor_copy(wt_sb[k][:, sl], wtp[:])

                psum_sq = psum_aux.tile([1, MC], FP32, name="psum_ssq")
                for j in range(KC):
                    ps = psum_main.tile([128, MC], FP32, name="psum_p1", tag="mm")
                    for k in range(KC):
                        nc.tensor.matmul(
                            ps[:],
                            lhsT=wt_sb[k][:, j * 128:(j + 1) * 128],
                            rhs=s_sb[k][:],
                            start=(k == 0),
                            stop=(k == KC - 1),
                        )
                    sp = sproj_pool.tile([128, MC], BF16, name=f"sproj{j}")
                    nc.scalar.activation(sp[:], ps[:], AF.Identity,
                                         bias=b_sb[j][:], scale=1.0)
                    sproj.append(sp)
                    # squared projection -> s_sq partial via ones-matmul
                    sq = sq_pool.tile([128, MC], BF16, name="sq")
                    nc.vector.tensor_mul(sq[:], sp[:], sp[:])
                    nc.tensor.matmul(
                        psum_sq[:],
                        lhsT=ones_col[:],
                        rhs=sq[:],
                        start=(j == 0),
                        stop=(j == KC - 1),
                    )

                # s_sq broadcast tile [128, MC] fp32 via log2-doubling DMAs
                ssq_bc = small_pool.tile([128, MC], FP32, name="ssq_bc")
                nc.scalar.activation(ssq_bc[0:1, :], psum_sq[:], AF.Identity)
                sh = 1
                while sh < 128:
                    nc.sync.dma_start(out=ssq_bc[sh:2 * sh, :],
                                      in_=ssq_bc[0:sh, :])
                    sh *= 2

            # ---- Phase 2: main GEMM over 32 row-chunks ----
            with (
                tc.tile_pool(name="tt", bufs=3 * KC) as t_pool,
                tc.tile_pool(name="ttb", bufs=3 * KC) as tb_pool,
                tc.tile_pool(name="osb", bufs=16) as out_pool,
                tc.tile_pool(name="obtp", bufs=4) as obt_pool,
                tc.tile_pool(name="tsq", bufs=3) as tsq_pool,
                tc.tile_pool(name="psum_gram", bufs=3, space="PSUM") as psum_gram,
            ):
                pending_out = []
                for g in range(NJ // 4):
                  tg_sb = []
                  for k in range(KC):
                      tt = t_pool.tile([128, 512], FP32, name="tt")
                      nc.sync.dma_start(out=tt[:], in_=t_in[g, k])
                      ttb = tb_pool.tile([128, 512], BF16, name="ttb")
                      nc.vector.tensor_copy(ttb[:], tt[:])
                      tg_sb.append(ttb)
                  # flush stores two groups back (sync stream stays waitless)
                  while len(pending_out) > 15:
                      oj, oob = pending_out.pop(0)
                      nc.sync.dma_start(out=out[oj], in_=oob[:])
                  for jj in range(4):
                    j = 4 * g + jj
                    t_sb = [tg_sb[k][:, jj * 128:(jj + 1) * 128] for k in range(KC)]

                    ps = psum_main.tile([128, MC], FP32, name="psum_main", tag="mm")
                    use_gram = variant in ("full", "gram", "gram_only", "gram_ttr")
                    use_ttr = variant in ("full", "gram", "gram_ttr")
                    use_bias = variant in ("full", "gram")
                    use_ssq = variant in ("full", "full_nogram")
                    for k in range(KC):
                        nc.tensor.matmul(
                            ps[:],
                            lhsT=t_sb[k],
                            rhs=sproj[k][:],
                            start=(k == 0),
                            stop=(k == KC - 1),
                        )
                        if use_gram:
                            if k == 0:
                                gram = psum_gram.tile([128, 128], FP32, name="psum_gram")
                            nc.tensor.matmul(
                                gram[:],
                                lhsT=t_sb[k],
                                rhs=t_sb[k],
                                start=(k == 0),
                                stop=(k == KC - 1),
                            )
                    ob = out_pool.tile([128, MC], FP32, name="osb")
                    if use_gram and not use_ttr:
                        # consume gram so it isn't dead: copy into scratch and DMA a row out
                        gsb = tsq_pool.tile([128, 128], FP32, name="gsb")
                        nc.scalar.activation(gsb[:], gram[:], AF.Identity)
                        nc.sync.dma_start(out=out[j][:, 0:128], in_=gsb[:])
                    if use_gram and use_ttr:
                        # t_sq[p] = sum_f gram[p, f] * I[p, f]
                        tsq = tsq_pool.tile([128, 1], FP32, name="tsq")
                        scratch = tsq_pool.tile([128, 128], FP32, name="tsq_scratch")
                        nc.vector.tensor_mul(scratch[:], gram[:], ident[:])
                        nc.vector.reduce_sum(tsq[:], scratch[:],
                                             axis=mybir.AxisListType.X)
                        # out = (-2 * cross + t_sq) + s_sq
                        obt = obt_pool.tile([128, MC], FP32, name="obt")
                        nc.scalar.activation(obt[:], ps[:], AF.Identity,
                                             bias=tsq[:], scale=-2.0)
                        nc.vector.tensor_add(ob[:], obt[:], ssq_bc[:])
                    else:
                        nc.scalar.activation(ob[:], ps[:], AF.Identity,
                                             scale=-2.0)
                    pending_out.append((j, ob))
                for (oj, oob) in pending_out:
                    nc.sync.dma_start(out=out[oj], in_=oob[:])

    nc.compile()
    return nc


_NC_CACHE = None


def _get_nc():
    global _NC_CACHE
    if _NC_CACHE is None:
        _NC_CACHE = build_nc()
    return _NC_CACHE


def stage_inputs(t_rep, s_rep, W, b):
    """Host-side layout staging (transpose/tile only) -> per-core input maps."""
    t_rep = np.asarray(t_rep, dtype=np.float32)
    s_rep = np.asarray(s_rep, dtype=np.float32)
    W = np.asarray(W, dtype=np.float32)
    b = np.asarray(b, dtype=np.float32)

    # t tiles: [NJ/4, KC, 128(d), 512(row)]; tile[g,k][p,c] = t_rep[g*512+c, k*128+p]
    t_tiles = np.ascontiguousarray(
        t_rep.reshape(NJ // 4, 512, KC, 128).transpose(0, 2, 3, 1)
    )
    # WT: [KC, 128, D]; WT[k][p, m] = W[m, k*128+p]
    wt = np.ascontiguousarray(W.T).reshape(KC, 128, D)
    b_st = np.ascontiguousarray(b.reshape(KC, 128, 1))

    in_maps = []
    for c in range(NCORES):
        s_slice = s_rep[c * MC:(c + 1) * MC]  # [512, D]
        # sT: [KC, 128, MC]; sT[k][p, r] = s_slice[r, k*128+p]
        s_st = np.ascontiguousarray(
            s_slice.reshape(MC, KC, 128).transpose(1, 2, 0)
        )
        in_maps.append({"t": t_tiles, "s": s_st, "w": wt, "b": b_st,
                        "ident": np.eye(128, dtype=np.float32)})
    return in_maps


def run_spmd(in_maps, **kwargs):
    nc = _get_nc()
    return run_bass_kernel_spmd(nc, in_maps, core_ids=list(range(NCORES)), **kwargs)


def gather_output(results):
    return np.concatenate(
        [results[c]["out"].reshape(N, MC) for c in range(NCORES)], axis=1
    )


def kernel(t_rep, s_rep, W, b):
    in_maps = stage_inputs(t_rep, s_rep, W, b)
    res = run_spmd(in_maps)
    return gather_output(res.results)



# revision 9
# speedup vs baseline: 1.9140x; 1.9140x over previous
"""Bass/Tile TRN2 kernel for retrieval-KNN MSE distance matrix.

Computes: out = ||t||^2 + ||s@W.T+b||^2 - 2 * t @ (s@W.T+b).T   [N=4096, M=4096]

Sharding (8 cores, output column-parallel, no collectives):
  core c holds s_rep rows [c*512, (c+1)*512) and computes the full-height
  output block out[:, c*512:(c+1)*512].

All matmuls run in fp8 e4m3 with DoubleRow perf mode (2 k-subtiles of 128
per instruction, 0.5 cycles/output-column). Inputs are cast to fp8 on the
host (pure precision staging; the same cast the device would do), which
also cuts HBM traffic 4x vs fp32. Error stays within budget because the
distance is computed consistently in quantized space:
    out = ||t8 - s'8||^2  (exact in fp32 accumulation)
  where t8 = fp8(t) and s'8 = fp8(-2*(s8@W8.T + b))/-2.

Per-core pipeline:
  GEMM1  : sproj_m2[d,c] = -2*(W8.T-chunks @ s8) + (-2b)   (fp8 out, ACT)
  s_sq   : ones-matmul over bf16 squares of sproj_m2, scaled 0.25,
           broadcast to [128,512] via log-doubling DMA.
  main   : psum = t8_j.T @ sproj_m2 (= -2*cross), 6 DoubleRow matmuls;
           gram = t8_j.T @ t8_j, diag -> t_sq via DVE tensor_tensor_reduce.
  epilog : out_fp16 = (psum + t_sq) + s_sq in one Pool scalar_tensor_tensor.
Output is fp16 (ulp ~1 at |out|~4e3), upcast to fp32 on host.
"""

import numpy as np
import ml_dtypes

import concourse.bacc as bacc
import concourse.bass as bass
import concourse.mybir as mybir
import concourse.tile as tile
from concourse.alu_op_type import AluOpType
from concourse.bass_utils import run_bass_kernel_spmd

N = 4096          # t_rep rows
M = 4096          # s_rep rows
D = 1536          # feature dim
NCORES = 8
MC = M // NCORES  # 512: output columns per core
KC = D // 128     # 12:  contraction chunks of 128
NJ = N // 128     # 32:  output row chunks per core
NG = NJ // 4      # 8:   512-row groups

FP32 = mybir.dt.float32
FP16 = mybir.dt.float16
BF16 = mybir.dt.bfloat16
FP8 = mybir.dt.float8e4
AF = mybir.ActivationFunctionType
DR = mybir.MatmulPerfMode.DoubleRow

NP_FP8 = ml_dtypes.float8_e4m3  # matches mybir.dt.np(dt.float8e4)

# "fast": DVE tensor_tensor_reduce + scalar_tensor_tensor epilogue
# "safe": baseline-proven ops only (tensor_mul/reduce_sum/activation/add)
EPILOG = "safe"


def build_nc():
    nc = bacc.Bacc("TRN2", target_bir_lowering=False, num_devices=NCORES)

    # host layouts are partition-major so each load is one big DMA
    t_in = nc.dram_tensor("t", [NG, 128, KC, 512], FP8, kind="ExternalInput").ap()
    s_in = nc.dram_tensor("s", [128, KC, MC], FP8, kind="ExternalInput").ap()
    w_in = nc.dram_tensor("w", [128, KC, D], FP8, kind="ExternalInput").ap()
    b_in = nc.dram_tensor("bneg2", [128, KC], FP32, kind="ExternalInput").ap()
    id_in = nc.dram_tensor("ident", [128, 128], FP32, kind="ExternalInput").ap()
    out = nc.dram_tensor("out", [NG, 128, 4, MC], FP16, kind="ExternalOutput").ap()

    with tile.TileContext(nc) as tc:
        with (
            tc.tile_pool(name="const", bufs=1) as const_pool,
            tc.tile_pool(name="wsb", bufs=1) as w_pool,
            tc.tile_pool(name="ssb", bufs=1) as s_pool,
            tc.tile_pool(name="sproj", bufs=1) as sproj_pool,
            tc.tile_pool(name="tsb", bufs=NG) as t_pool,
            tc.tile_pool(name="sq", bufs=KC) as sq_pool,
            tc.tile_pool(name="tsq", bufs=4) as tsq_pool,
            tc.tile_pool(name="scr", bufs=2) as scr_pool,
            tc.tile_pool(name="osb", bufs=3) as out_pool,
            tc.tile_pool(name="tmp", bufs=3) as tmp_pool,
            tc.tile_pool(name="psum_main", bufs=4, space="PSUM") as psum_main,
            tc.tile_pool(name="psum_gram", bufs=2, space="PSUM") as psum_gram,
            tc.tile_pool(name="psum_ssq", bufs=1, space="PSUM") as psum_ssq,
        ):
            # ---- input loads: GEMM1 operands first (critical path) ----
            ssb = s_pool.tile([128, KC, MC], FP8)
            nc.sync.dma_start(out=ssb[:], in_=s_in[:])
            wsb = w_pool.tile([128, KC, D], FP8)
            nc.sync.dma_start(out=wsb[:], in_=w_in[:])
            bsb = const_pool.tile([128, KC], FP32)
            nc.sync.dma_start(out=bsb[:], in_=b_in[:])
            ident = const_pool.tile([128, 128], FP32)
            nc.sync.dma_start(out=ident[:], in_=id_in[:])
            tsb = []
            for g in range(NG):
                tt = t_pool.tile([128, KC, 512], FP8, name="tsb")
                nc.sync.dma_start(out=tt[:], in_=t_in[g])
                tsb.append(tt)

            ones_k = const_pool.tile([128, 1], BF16)
            nc.vector.memset(ones_k[:], 1.0)

            # ---- PE p-state warmup while initial DMAs stream ----
            warm = const_pool.tile([128, 2, 512], FP8, name="warm")
            nc.vector.memset(warm[:], 0.5)
            with tc.tile_pool(name="psum_warm", bufs=1, space="PSUM") as pw_pool:
                pw = pw_pool.tile([128, MC], FP32, name="pw")
                for i in range(20):
                    nc.tensor.matmul(pw[:], lhsT=warm[:, :, 0:128], rhs=warm[:],
                                     start=(i == 0), stop=(i == 19),
                                     perf_mode=DR)

            # ---- GEMM1: sproj_m2[d, c] = -2*(proj + b), fp8 ----
            sproj = sproj_pool.tile([128, KC, MC], FP8)
            sq_tiles = []
            for jc in range(KC):
                ps = psum_main.tile([128, MC], FP32, name="psum_g1", tag="mm")
                for a in range(KC // 2):
                    nc.tensor.matmul(
                        ps[:],
                        lhsT=wsb[:, 2 * a:2 * a + 2, jc * 128:(jc + 1) * 128],
                        rhs=ssb[:, 2 * a:2 * a + 2, :],
                        start=(a == 0),
                        stop=(a == KC // 2 - 1),
                        perf_mode=DR,
                    )
                nc.scalar.activation(sproj[:, jc, :], ps[:], AF.Identity,
                                     bias=bsb[:, jc:jc + 1], scale=-2.0)
                sq = sq_pool.tile([128, MC], BF16, name="sq")
                nc.vector.tensor_mul(sq[:], sproj[:, jc, :], sproj[:, jc, :])
                sq_tiles.append(sq)

            # s_sq partials: ones-matmul over squared (-2s')  -> 4*s_sq [1,MC]
            psum_sq = psum_ssq.tile([1, MC], FP32, name="psum_ssq")
            for jc in range(KC):
                nc.tensor.matmul(
                    psum_sq[:],
                    lhsT=ones_k[:],
                    rhs=sq_tiles[jc][:],
                    start=(jc == 0),
                    stop=(jc == KC - 1),
                )
            # broadcast 0.25*psum_sq to [128, MC] via log-doubling DMAs
            # (fp32 for the DVE epilogue, fp16 for the Pool epilogue)
            ssq_bc = const_pool.tile([128, MC], FP32, name="ssq_bc")
            nc.scalar.activation(ssq_bc[0:1, :], psum_sq[:], AF.Identity,
                                 scale=0.25)
            ssq_bc16 = const_pool.tile([128, MC], FP16, name="ssq_bc16")
            nc.scalar.activation(ssq_bc16[0:1, :], psum_sq[:], AF.Identity,
                                 scale=0.25)
            sh = 1
            while sh < 128:
                nc.sync.dma_start(out=ssq_bc[sh:2 * sh, :], in_=ssq_bc[0:sh, :])
                nc.sync.dma_start(out=ssq_bc16[sh:2 * sh, :],
                                  in_=ssq_bc16[0:sh, :])
                sh *= 2

            # ---- main loop over 32 row-chunks ----
            for g in range(NG):
                ob = out_pool.tile([128, 4, MC], FP16, name="osb")
                for jj in range(4):
                    ps = psum_main.tile([128, MC], FP32, name="psum_mm", tag="mm")
                    gram = psum_gram.tile([128, 128], FP32, name="psum_gram")
                    rsl = slice(jj * 128, (jj + 1) * 128)
                    for a in range(KC // 2):
                        ksl = slice(2 * a, 2 * a + 2)
                        nc.tensor.matmul(
                            ps[:],
                            lhsT=tsb[g][:, ksl, rsl],
                            rhs=sproj[:, ksl, :],
                            start=(a == 0),
                            stop=(a == KC // 2 - 1),
                            perf_mode=DR,
                        )
                        nc.tensor.matmul(
                            gram[:],
                            lhsT=tsb[g][:, ksl, rsl],
                            rhs=tsb[g][:, ksl, rsl],
                            start=(a == 0),
                            stop=(a == KC // 2 - 1),
                            perf_mode=DR,
                        )
                    # t_sq[p] = sum_f gram[p,f] * I[p,f]
                    tsq = tsq_pool.tile([128, 1], FP32, name="tsq")
                    if EPILOG == "fast":
                        scr = scr_pool.tile([128, 128], FP32, name="scr")
                        nc.vector.tensor_tensor_reduce(
                            out=scr[:], in0=gram[:], in1=ident[:], scale=1.0,
                            scalar=0.0, op0=AluOpType.mult, op1=AluOpType.add,
                            accum_out=tsq[:],
                        )
                    else:
                        scr = scr_pool.tile([128, 128], FP32, name="scr")
                        nc.vector.tensor_mul(scr[:], gram[:], ident[:])
                        nc.vector.reduce_sum(tsq[:], scr[:],
                                             axis=mybir.AxisListType.X)
                    # out = (psum + t_sq) + s_sq   (psum = -2*cross)
                    # Pool can't touch PSUM: even jj go ACT(+t_sq) ->
                    # Pool(+s_sq, SBUF only); odd jj go DVE in one STT.
                    if EPILOG == "fast" and jj % 2 == 1:
                        nc.vector.scalar_tensor_tensor(
                            out=ob[:, jj, :], in0=ps[:], scalar=tsq[:],
                            in1=ssq_bc[:], op0=AluOpType.add,
                            op1=AluOpType.add,
                        )
                    else:
                        tmp = tmp_pool.tile([128, MC], FP16, name="tmp")
                        nc.scalar.activation(tmp[:], ps[:], AF.Identity,
                                             bias=tsq[:], scale=1.0)
                        if EPILOG == "fast":
                            nc.gpsimd.tensor_add(ob[:, jj, :], tmp[:],
                                                 ssq_bc16[:])
                        else:
                            nc.vector.tensor_add(ob[:, jj, :], tmp[:],
                                                 ssq_bc16[:])
                nc.sync.dma_start(out=out[g], in_=ob[:])

    nc.compile()
    return nc


_NC_CACHE = None


def _get_nc():
    global _NC_CACHE
    if _NC_CACHE is None:
        _NC_CACHE = build_nc()
    return _NC_CACHE


def stage_inputs(t_rep, s_rep, W, b):
    """Host-side layout + precision staging -> per-core input maps."""
    t_rep = np.asarray(t_rep, dtype=np.float32)
    s_rep = np.asarray(s_rep, dtype=np.float32)
    W = np.asarray(W, dtype=np.float32)
    b = np.asarray(b, dtype=np.float32)

    # t8[g, p, k, r] = t[g*512 + r, k*128 + p]
    t8 = np.ascontiguousarray(
        t_rep.reshape(NG, 512, KC, 128).transpose(0, 3, 2, 1)
    ).astype(NP_FP8)
    # w8[p, k, m] = W[m, k*128 + p]
    w8 = np.ascontiguousarray(
        W.reshape(D, KC, 128).transpose(2, 1, 0)
    ).astype(NP_FP8)
    # bneg2[p, k] = -2*b[k*128+p]
    bneg2 = np.ascontiguousarray((-2.0 * b).reshape(KC, 128).T)
    ident = np.eye(128, dtype=np.float32)

    in_maps = []
    for c in range(NCORES):
        s_slice = s_rep[c * MC:(c + 1) * MC]  # [512, D]
        # s8[p, k, r] = s_slice[r, k*128 + p]
        s8 = np.ascontiguousarray(
            s_slice.reshape(MC, KC, 128).transpose(2, 1, 0)
        ).astype(NP_FP8)
        in_maps.append({"t": t8, "s": s8, "w": w8, "bneg2": bneg2,
                        "ident": ident})
    return in_maps


def run_spmd(in_maps, **kwargs):
    nc = _get_nc()
    return run_bass_kernel_spmd(nc, in_maps, core_ids=list(range(NCORES)), **kwargs)


def gather_output(results):
    cols = []
    for c in range(NCORES):
        o = np.asarray(results[c]["out"])  # [NG, 128, 4, MC] fp16
        cols.append(o.transpose(0, 2, 1, 3).reshape(N, MC).astype(np.float32))
    return np.concatenate(cols, axis=1)


def kernel(t_rep, s_rep, W, b):
    in_maps = stage_inputs(t_rep, s_rep, W, b)
    res = run_spmd(in_maps)
    return gather_output(res.results)


# revision 11
# speedup vs baseline: 1.9631x; 1.0257x over previous
"""Bass/Tile TRN2 kernel for retrieval-KNN MSE distance matrix.

Computes: out = ||t||^2 + ||s@W.T+b||^2 - 2 * t @ (s@W.T+b).T   [N=4096, M=4096]

Sharding (8 cores, output column-parallel, no collectives):
  core c holds s_rep rows [c*512, (c+1)*512) and computes the full-height
  output block out[:, c*512:(c+1)*512].

All matmuls run in fp8 e4m3 with DoubleRow perf mode (two k-subtiles of
128 per instruction, 2x bf16 throughput). Inputs are cast to fp8 on the
host (the same cast the device would otherwise do), which also cuts HBM
traffic 4x vs fp32. Error stays in budget because the distance is
computed consistently in quantized space: out = ||t8 - s'8||^2 exactly
(fp32 accumulation), t8 = fp8(t), s'8 = fp8(-2(s8@W8.T+b))/-2.

Per-core pipeline:
  loads  : s, then W in 4 column-chunks (GEMM1 starts after chunk 0),
           then the 8 t row-groups; everything is SBUF-resident once.
  GEMM1  : sproj_m2[d,c] = -2*(W8.T @ s8 + b) in fp8 (ACT copyback);
           squares (DVE) and s_sq ones-matmuls interleaved, staggered 3
           behind so the PE never waits on them.
  s_sq   : [1,512] row scaled 0.25 (ACT) -> K=1 ones matmul broadcast
           to [128,512] -> fp32 + fp16 SBUF copies (ACT).
  main   : psum = t8_j.T @ sproj_m2 (= -2*cross) + gram = t8_j.T @ t8_j,
           6 DoubleRow matmuls each, interleaved; t_sq = diag(gram) via
           DVE mask-mul + reduce (tensor_tensor_reduce hangs TRN2 hw).
  epilog : ACT tmp16 = psum + t_sq (bias add), Pool ob = tmp16 + s_sq;
           fp16 stores batched 4 row-chunks per DMA.
Output fp16 (ulp ~1 at |out|~4e3), upcast to fp32 on host.
"""

import numpy as np
import ml_dtypes

import concourse.bacc as bacc
import concourse.bass as bass
import concourse.mybir as mybir
import concourse.tile as tile
from concourse.alu_op_type import AluOpType
from concourse.bass_utils import run_bass_kernel_spmd

N = 4096          # t_rep rows
M = 4096          # s_rep rows
D = 1536          # feature dim
NCORES = 8
MC = M // NCORES  # 512: output columns per core
KC = D // 128     # 12:  contraction chunks of 128
NJ = N // 128     # 32:  output row chunks per core
NG = NJ // 4      # 8:   512-row groups
WCH = 4           # W column chunks
WCOLS = D // WCH  # 384 cols per W chunk

FP32 = mybir.dt.float32
FP16 = mybir.dt.float16
BF16 = mybir.dt.bfloat16
FP8 = mybir.dt.float8e4
AF = mybir.ActivationFunctionType
DR = mybir.MatmulPerfMode.DoubleRow

NP_FP8 = ml_dtypes.float8_e4m3  # matches mybir.dt.np(dt.float8e4)

N_WARM = 24       # PE p-state warmup matmuls


def build_nc():
    nc = bacc.Bacc("TRN2", target_bir_lowering=False, num_devices=NCORES)

    # host layouts are partition-major so each load is one big DMA
    t_in = nc.dram_tensor("t", [NG, 128, KC, 512], FP8, kind="ExternalInput").ap()
    s_in = nc.dram_tensor("s", [128, KC, MC], FP8, kind="ExternalInput").ap()
    w_in = nc.dram_tensor("w", [WCH, 128, KC, WCOLS], FP8,
                          kind="ExternalInput").ap()
    b_in = nc.dram_tensor("bneg2", [128, KC], FP32, kind="ExternalInput").ap()
    id_in = nc.dram_tensor("ident", [128, 128], FP32, kind="ExternalInput").ap()
    out = nc.dram_tensor("out", [NG, 128, 4, MC], FP16, kind="ExternalOutput").ap()

    with tile.TileContext(nc) as tc:
        with (
            tc.tile_pool(name="const", bufs=1) as const_pool,
            tc.tile_pool(name="wsb", bufs=WCH) as w_pool,
            tc.tile_pool(name="ssb", bufs=1) as s_pool,
            tc.tile_pool(name="sproj", bufs=1) as sproj_pool,
            tc.tile_pool(name="tsb", bufs=NG) as t_pool,
            tc.tile_pool(name="sq", bufs=KC) as sq_pool,
            tc.tile_pool(name="tsq", bufs=4) as tsq_pool,
            tc.tile_pool(name="scr", bufs=2) as scr_pool,
            tc.tile_pool(name="osb", bufs=3) as out_pool,
            tc.tile_pool(name="tmp", bufs=3) as tmp_pool,
            tc.tile_pool(name="psum_main", bufs=4, space="PSUM") as psum_main,
            tc.tile_pool(name="psum_gram", bufs=2, space="PSUM") as psum_gram,
            tc.tile_pool(name="psum_ssq", bufs=1, space="PSUM") as psum_ssq,
        ):
            # ---- input loads: GEMM1 operands first (critical path) ----
            ssb = s_pool.tile([128, KC, MC], FP8)
            nc.sync.dma_start(out=ssb[:], in_=s_in[:])
            wsb = []
            for c in range(WCH):
                wt = w_pool.tile([128, KC, WCOLS], FP8, name="wsb")
                nc.sync.dma_start(out=wt[:], in_=w_in[c])
                wsb.append(wt)
            bsb = const_pool.tile([128, KC], FP32)
            nc.sync.dma_start(out=bsb[:], in_=b_in[:])
            ident = const_pool.tile([128, 128], FP32)
            nc.sync.dma_start(out=ident[:], in_=id_in[:])
            tsb = []
            for g in range(NG):
                tt = t_pool.tile([128, KC, 512], FP8, name="tsb")
                nc.sync.dma_start(out=tt[:], in_=t_in[g])
                tsb.append(tt)

            ones_k = const_pool.tile([128, 1], BF16)
            nc.vector.memset(ones_k[:], 1.0)
            ones_1 = const_pool.tile([1, 128], FP16)
            nc.vector.memset(ones_1[:], 1.0)

            # ---- PE p-state warmup while initial DMAs stream ----
            warm = const_pool.tile([128, 2, 512], FP8, name="warm")
            nc.vector.memset(warm[:], 0.5)
            pw = psum_main.tile([128, MC], FP32, name="pw", tag="mm")
            for i in range(N_WARM):
                nc.tensor.matmul(pw[:], lhsT=warm[:, :, 0:128], rhs=warm[:],
                                 start=(i == 0), stop=(i == N_WARM - 1),
                                 perf_mode=DR)

            # ---- GEMM1 + interleaved s_sq reduction ----
            sproj = sproj_pool.tile([128, KC, MC], FP8)
            sq_tiles = []
            psum_sq = psum_ssq.tile([1, MC], FP32, name="psum_ssq")

            def ssq_mm(jc):
                nc.tensor.matmul(
                    psum_sq[:], lhsT=ones_k[:], rhs=sq_tiles[jc][:],
                    start=(jc == 0), stop=(jc == KC - 1),
                )

            for jc in range(KC):
                ps = psum_main.tile([128, MC], FP32, name="psum_g1", tag="mm")
                wt = wsb[jc // (KC // WCH)]
                col = (jc % (KC // WCH)) * 128
                for a in range(KC // 2):
                    nc.tensor.matmul(
                        ps[:],
                        lhsT=wt[:, 2 * a:2 * a + 2, col:col + 128],
                        rhs=ssb[:, 2 * a:2 * a + 2, :],
                        start=(a == 0),
                        stop=(a == KC // 2 - 1),
                        perf_mode=DR,
                    )
                if jc >= 3:
                    ssq_mm(jc - 3)  # staggered so the PE never waits
                nc.scalar.activation(sproj[:, jc, :], ps[:], AF.Identity,
                                     bias=bsb[:, jc:jc + 1], scale=-2.0)
                sq = sq_pool.tile([128, MC], BF16, name="sq")
                nc.vector.tensor_mul(sq[:], sproj[:, jc, :], sproj[:, jc, :])
                sq_tiles.append(sq)
            for jc in range(KC - 3, KC):
                ssq_mm(jc)

            # s_sq row (0.25x) then K=1 ones-matmul broadcast to [128, MC]
            ssq_row = const_pool.tile([1, MC], FP16, name="ssq_row")
            nc.scalar.activation(ssq_row[:], psum_sq[:], AF.Identity,
                                 scale=0.25)
            ssq_bc16 = const_pool.tile([128, MC], FP16, name="ssq_bc16")

            # ---- main loop over 32 row-chunks ----
            first = True
            for g in range(NG):
                ob = out_pool.tile([128, 4, MC], FP16, name="osb")
                for jj in range(4):
                    ps = psum_main.tile([128, MC], FP32, name="psum_mm", tag="mm")
                    gram = psum_gram.tile([128, 128], FP32, name="psum_gram")
                    rsl = slice(jj * 128, (jj + 1) * 128)
                    for a in range(KC // 2):
                        ksl = slice(2 * a, 2 * a + 2)
                        nc.tensor.matmul(
                            ps[:],
                            lhsT=tsb[g][:, ksl, rsl],
                            rhs=sproj[:, ksl, :],
                            start=(a == 0),
                            stop=(a == KC // 2 - 1),
                            perf_mode=DR,
                        )
                        nc.tensor.matmul(
                            gram[:],
                            lhsT=tsb[g][:, ksl, rsl],
                            rhs=tsb[g][:, ksl, rsl],
                            start=(a == 0),
                            stop=(a == KC // 2 - 1),
                            perf_mode=DR,
                        )
                    if first:
                        # broadcast matmul placed after j0's matmuls so the
                        # PE doesn't stall waiting for the s_sq row
                        bps = psum_ssq.tile([128, MC], FP32, name="psum_bc",
                                            bufs=1)
                        nc.tensor.matmul(bps[:], lhsT=ones_1[:], rhs=ssq_row[:],
                                         start=True, stop=True)
                        nc.scalar.activation(ssq_bc16[:], bps[:], AF.Identity)
                        first = False
                    # t_sq[p] = sum_f gram[p,f] * I[p,f]
                    tsq = tsq_pool.tile([128, 1], FP32, name="tsq")
                    scr = scr_pool.tile([128, 128], FP32, name="scr")
                    nc.vector.tensor_mul(scr[:], gram[:], ident[:])
                    nc.vector.reduce_sum(tsq[:], scr[:],
                                         axis=mybir.AxisListType.X)
                    # out = (psum + t_sq) + s_sq   (psum = -2*cross)
                    tmp = tmp_pool.tile([128, MC], FP16, name="tmp")
                    nc.scalar.activation(tmp[:], ps[:], AF.Identity,
                                         bias=tsq[:], scale=1.0)
                    nc.gpsimd.tensor_add(ob[:, jj, :], tmp[:], ssq_bc16[:])
                nc.sync.dma_start(out=out[g], in_=ob[:])

    nc.compile()
    return nc


_NC_CACHE = None


def _get_nc():
    global _NC_CACHE
    if _NC_CACHE is None:
        _NC_CACHE = build_nc()
    return _NC_CACHE


def stage_inputs(t_rep, s_rep, W, b):
    """Host-side layout + precision staging -> per-core input maps."""
    t_rep = np.asarray(t_rep, dtype=np.float32)
    s_rep = np.asarray(s_rep, dtype=np.float32)
    W = np.asarray(W, dtype=np.float32)
    b = np.asarray(b, dtype=np.float32)

    # t8[g, p, k, r] = t[g*512 + r, k*128 + p]
    t8 = np.ascontiguousarray(
        t_rep.reshape(NG, 512, KC, 128).transpose(0, 3, 2, 1)
    ).astype(NP_FP8)
    # w8[c, p, k, m] = W[c*384 + m, k*128 + p]
    w8 = np.ascontiguousarray(
        W.reshape(WCH, WCOLS, KC, 128).transpose(0, 3, 2, 1)
    ).astype(NP_FP8)
    # bneg2[p, k] = -2*b[k*128+p]
    bneg2 = np.ascontiguousarray((-2.0 * b).reshape(KC, 128).T)
    ident = np.eye(128, dtype=np.float32)

    in_maps = []
    for c in range(NCORES):
        s_slice = s_rep[c * MC:(c + 1) * MC]  # [512, D]
        # s8[p, k, r] = s_slice[r, k*128 + p]
        s8 = np.ascontiguousarray(
            s_slice.reshape(MC, KC, 128).transpose(2, 1, 0)
        ).astype(NP_FP8)
        in_maps.append({"t": t8, "s": s8, "w": w8, "bneg2": bneg2,
                        "ident": ident})
    return in_maps


def run_spmd(in_maps, **kwargs):
    nc = _get_nc()
    return run_bass_kernel_spmd(nc, in_maps, core_ids=list(range(NCORES)), **kwargs)


def gather_output(results):
    cols = []
    for c in range(NCORES):
        o = np.asarray(results[c]["out"])  # [NG, 128, 4, MC] fp16
        cols.append(o.transpose(0, 2, 1, 3).reshape(N, MC).astype(np.float32))
    return np.concatenate(cols, axis=1)


def kernel(t_rep, s_rep, W, b):
    in_maps = stage_inputs(t_rep, s_rep, W, b)
    res = run_spmd(in_maps)
    return gather_output(res.results)


# revision 12
# speedup vs baseline: 1.9896x; 1.0135x over previous
"""Bass/Tile TRN2 kernel for retrieval-KNN MSE distance matrix.

Computes: out = ||t||^2 + ||s@W.T+b||^2 - 2 * t @ (s@W.T+b).T   [N=4096, M=4096]

Sharding (8 cores, output column-parallel, no collectives):
  core c holds s_rep rows [c*512, (c+1)*512) and computes the full-height
  output block out[:, c*512:(c+1)*512].

All matmuls run in fp8 e4m3 with DoubleRow perf mode (two k-subtiles of
128 per instruction, 2x bf16 throughput). Inputs are cast to fp8 on the
host (the same cast the device would otherwise do), which also cuts HBM
traffic 4x vs fp32. Error stays in budget because the distance is
computed consistently in quantized space: out = ||t8 - s'8||^2 exactly
(fp32 accumulation), t8 = fp8(t), s'8 = fp8(-2(s8@W8.T+b))/-2.

Per-core pipeline:
  loads  : s, then W in 4 column-chunks (GEMM1 starts after chunk 0),
           then the 8 t row-groups; everything is SBUF-resident once.
  GEMM1  : sproj_m2[d,c] = -2*(W8.T @ s8 + b) in fp8 (ACT copyback);
           squares (DVE) and s_sq ones-matmuls interleaved, staggered 3
           behind so the PE never waits on them.
  s_sq   : [1,512] row scaled 0.25 (ACT) -> K=1 ones matmul broadcast
           to [128,512] -> fp32 + fp16 SBUF copies (ACT).
  main   : psum = t8_j.T @ sproj_m2 (= -2*cross) + gram = t8_j.T @ t8_j,
           6 DoubleRow matmuls each, interleaved; t_sq = diag(gram) via
           DVE mask-mul + reduce (tensor_tensor_reduce hangs TRN2 hw).
  epilog : ACT tmp16 = psum + t_sq (bias add), Pool ob = tmp16 + s_sq;
           fp16 stores batched 4 row-chunks per DMA.
Output fp16 (ulp ~1 at |out|~4e3), upcast to fp32 on host.
"""

import numpy as np
import ml_dtypes

import concourse.bacc as bacc
import concourse.bass as bass
import concourse.mybir as mybir
import concourse.tile as tile
from concourse.alu_op_type import AluOpType
from concourse.bass_utils import run_bass_kernel_spmd

N = 4096          # t_rep rows
M = 4096          # s_rep rows
D = 1536          # feature dim
NCORES = 8
MC = M // NCORES  # 512: output columns per core
KC = D // 128     # 12:  contraction chunks of 128
NJ = N // 128     # 32:  output row chunks per core
NG = NJ // 4      # 8:   512-row groups
WCH = 4           # W column chunks
WCOLS = D // WCH  # 384 cols per W chunk

FP32 = mybir.dt.float32
FP16 = mybir.dt.float16
BF16 = mybir.dt.bfloat16
FP8 = mybir.dt.float8e4
AF = mybir.ActivationFunctionType
DR = mybir.MatmulPerfMode.DoubleRow

NP_FP8 = ml_dtypes.float8_e4m3  # matches mybir.dt.np(dt.float8e4)

N_WARM = 24       # PE p-state warmup matmuls


def build_nc():
    nc = bacc.Bacc("TRN2", target_bir_lowering=False, num_devices=NCORES)

    # host layouts are partition-major so each load is one big DMA
    t_in = nc.dram_tensor("t", [NG, 128, KC, 512], FP8, kind="ExternalInput").ap()
    s_in = nc.dram_tensor("s", [128, KC, MC], FP8, kind="ExternalInput").ap()
    w_in = nc.dram_tensor("w", [WCH, 128, KC, WCOLS], FP8,
                          kind="ExternalInput").ap()
    b_in = nc.dram_tensor("bneg2", [128, KC], FP32, kind="ExternalInput").ap()
    id_in = nc.dram_tensor("ident", [128, 128], FP32, kind="ExternalInput").ap()
    out = nc.dram_tensor("out", [NG, 128, 4, MC], FP16, kind="ExternalOutput").ap()

    with tile.TileContext(nc) as tc:
        with (
            tc.tile_pool(name="const", bufs=1) as const_pool,
            tc.tile_pool(name="wsb", bufs=WCH) as w_pool,
            tc.tile_pool(name="ssb", bufs=1) as s_pool,
            tc.tile_pool(name="sproj", bufs=1) as sproj_pool,
            tc.tile_pool(name="tsb", bufs=NG) as t_pool,
            tc.tile_pool(name="sq", bufs=KC) as sq_pool,
            tc.tile_pool(name="tsq", bufs=4) as tsq_pool,
            tc.tile_pool(name="scr", bufs=2) as scr_pool,
            tc.tile_pool(name="osb", bufs=3) as out_pool,
            tc.tile_pool(name="tmp", bufs=3) as tmp_pool,
            tc.tile_pool(name="psum_main", bufs=4, space="PSUM") as psum_main,
            tc.tile_pool(name="psum_gram", bufs=2, space="PSUM") as psum_gram,
            tc.tile_pool(name="psum_ssq", bufs=1, space="PSUM") as psum_ssq,
        ):
            # ---- input loads: GEMM1 operands first (critical path) ----
            ssb = s_pool.tile([128, KC, MC], FP8)
            nc.sync.dma_start(out=ssb[:], in_=s_in[:])
            wsb = []
            for c in range(WCH):
                wt = w_pool.tile([128, KC, WCOLS], FP8, name="wsb")
                nc.sync.dma_start(out=wt[:], in_=w_in[c])
                wsb.append(wt)
            bsb = const_pool.tile([128, KC], FP32)
            nc.sync.dma_start(out=bsb[:], in_=b_in[:])
            ident = const_pool.tile([128, 128], FP32)
            nc.sync.dma_start(out=ident[:], in_=id_in[:])
            tsb = []
            for g in range(NG):
                tt = t_pool.tile([128, KC, 512], FP8, name="tsb")
                nc.sync.dma_start(out=tt[:], in_=t_in[g])
                tsb.append(tt)

            ones_k = const_pool.tile([128, 1], BF16)
            nc.vector.memset(ones_k[:], 1.0)
            ones_1 = const_pool.tile([1, 128], FP16)
            nc.vector.memset(ones_1[:], 1.0)

            # ---- PE p-state warmup while initial DMAs stream ----
            warm = const_pool.tile([128, 2, 512], FP8, name="warm")
            nc.vector.memset(warm[:], 0.5)
            pw = psum_main.tile([128, MC], FP32, name="pw", tag="mm")
            for i in range(N_WARM):
                nc.tensor.matmul(pw[:], lhsT=warm[:, :, 0:128], rhs=warm[:],
                                 start=(i == 0), stop=(i == N_WARM - 1),
                                 perf_mode=DR)

            # ---- GEMM1 + interleaved s_sq reduction ----
            sproj = sproj_pool.tile([128, KC, MC], FP8)
            sq_tiles = []
            psum_sq = psum_ssq.tile([1, MC], FP32, name="psum_ssq")

            def ssq_mm(jc):
                nc.tensor.matmul(
                    psum_sq[:], lhsT=ones_k[:], rhs=sq_tiles[jc][:],
                    start=(jc == 0), stop=(jc == KC - 1),
                )

            for jc in range(KC):
                ps = psum_main.tile([128, MC], FP32, name="psum_g1", tag="mm")
                wt = wsb[jc // (KC // WCH)]
                col = (jc % (KC // WCH)) * 128
                for a in range(KC // 2):
                    nc.tensor.matmul(
                        ps[:],
                        lhsT=wt[:, 2 * a:2 * a + 2, col:col + 128],
                        rhs=ssb[:, 2 * a:2 * a + 2, :],
                        start=(a == 0),
                        stop=(a == KC // 2 - 1),
                        perf_mode=DR,
                    )
                if jc >= 3:
                    ssq_mm(jc - 3)  # staggered so the PE never waits
                nc.scalar.activation(sproj[:, jc, :], ps[:], AF.Identity,
                                     bias=bsb[:, jc:jc + 1], scale=-2.0)
                sq = sq_pool.tile([128, MC], BF16, name="sq")
                nc.vector.tensor_mul(sq[:], sproj[:, jc, :], sproj[:, jc, :])
                sq_tiles.append(sq)
            for jc in range(KC - 3, KC):
                ssq_mm(jc)

            # s_sq row (0.25x) then K=1 ones-matmul broadcast to [128, MC]
            ssq_row = const_pool.tile([1, MC], FP16, name="ssq_row")
            nc.scalar.activation(ssq_row[:], psum_sq[:], AF.Identity,
                                 scale=0.25)
            ssq_bc16 = const_pool.tile([128, MC], FP16, name="ssq_bc16")

            # ---- main loop over 32 row-chunks ----
            first = True
            for g in range(NG):
                ob = out_pool.tile([128, 4, MC], FP16, name="osb")
                for jj in range(4):
                    ps = psum_main.tile([128, MC], FP32, name="psum_mm", tag="mm")
                    gram = psum_gram.tile([128, 128], FP32, name="psum_gram")
                    rsl = slice(jj * 128, (jj + 1) * 128)
                    for a in range(KC // 2):
                        ksl = slice(2 * a, 2 * a + 2)
                        # gram first: it shares lhsT with the main matmul, so
                        # the main matmul's weight load can be skipped/hidden
                        nc.tensor.matmul(
                            gram[:],
                            lhsT=tsb[g][:, ksl, rsl],
                            rhs=tsb[g][:, ksl, rsl],
                            start=(a == 0),
                            stop=(a == KC // 2 - 1),
                            perf_mode=DR,
                        )
                        nc.tensor.matmul(
                            ps[:],
                            lhsT=tsb[g][:, ksl, rsl],
                            rhs=sproj[:, ksl, :],
                            start=(a == 0),
                            stop=(a == KC // 2 - 1),
                            perf_mode=DR,
                        )
                    if first:
                        # broadcast matmul placed after j0's matmuls so the
                        # PE doesn't stall waiting for the s_sq row
                        bps = psum_ssq.tile([128, MC], FP32, name="psum_bc",
                                            bufs=1)
                        nc.tensor.matmul(bps[:], lhsT=ones_1[:], rhs=ssq_row[:],
                                         start=True, stop=True)
                        nc.scalar.activation(ssq_bc16[:], bps[:], AF.Identity)
                        first = False
                    # t_sq[p] = sum_f gram[p,f] * I[p,f]
                    tsq = tsq_pool.tile([128, 1], FP32, name="tsq")
                    scr = scr_pool.tile([128, 128], FP32, name="scr")
                    nc.vector.tensor_mul(scr[:], gram[:], ident[:])
                    nc.vector.reduce_sum(tsq[:], scr[:],
                                         axis=mybir.AxisListType.X)
                    # out = (psum + t_sq) + s_sq   (psum = -2*cross)
                    tmp = tmp_pool.tile([128, MC], FP16, name="tmp")
                    nc.scalar.activation(tmp[:], ps[:], AF.Identity,
                                         bias=tsq[:], scale=1.0)
                    nc.gpsimd.tensor_add(ob[:, jj, :], tmp[:], ssq_bc16[:])
                nc.sync.dma_start(out=out[g], in_=ob[:])

    nc.compile()
    return nc


_NC_CACHE = None


def _get_nc():
    global _NC_CACHE
    if _NC_CACHE is None:
        _NC_CACHE = build_nc()
    return _NC_CACHE


def stage_inputs(t_rep, s_rep, W, b):
    """Host-side layout + precision staging -> per-core input maps."""
    t_rep = np.asarray(t_rep, dtype=np.float32)
    s_rep = np.asarray(s_rep, dtype=np.float32)
    W = np.asarray(W, dtype=np.float32)
    b = np.asarray(b, dtype=np.float32)

    # t8[g, p, k, r] = t[g*512 + r, k*128 + p]
    t8 = np.ascontiguousarray(
        t_rep.reshape(NG, 512, KC, 128).transpose(0, 3, 2, 1)
    ).astype(NP_FP8)
    # w8[c, p, k, m] = W[c*384 + m, k*128 + p]
    w8 = np.ascontiguousarray(
        W.reshape(WCH, WCOLS, KC, 128).transpose(0, 3, 2, 1)
    ).astype(NP_FP8)
    # bneg2[p, k] = -2*b[k*128+p]
    bneg2 = np.ascontiguousarray((-2.0 * b).reshape(KC, 128).T)
    ident = np.eye(128, dtype=np.float32)

    in_maps = []
    for c in range(NCORES):
        s_slice = s_rep[c * MC:(c + 1) * MC]  # [512, D]
        # s8[p, k, r] = s_slice[r, k*128 + p]
        s8 = np.ascontiguousarray(
            s_slice.reshape(MC, KC, 128).transpose(2, 1, 0)
        ).astype(NP_FP8)
        in_maps.append({"t": t8, "s": s8, "w": w8, "bneg2": bneg2,
                        "ident": ident})
    return in_maps


def run_spmd(in_maps, **kwargs):
    nc = _get_nc()
    return run_bass_kernel_spmd(nc, in_maps, core_ids=list(range(NCORES)), **kwargs)


def gather_output(results):
    cols = []
    for c in range(NCORES):
        o = np.asarray(results[c]["out"])  # [NG, 128, 4, MC] fp16
        cols.append(o.transpose(0, 2, 1, 3).reshape(N, MC).astype(np.float32))
    return np.concatenate(cols, axis=1)


def kernel(t_rep, s_rep, W, b):
    in_maps = stage_inputs(t_rep, s_rep, W, b)
    res = run_spmd(in_maps)
    return gather_output(res.results)
